# revision 23
# baseline (speedup 1.0000x reference)
"""Balanced BCE loss on 8 Trainium2 NeuronCores.

loss = -sum_i [ beta_i * sum_j(t_ij * ln(p_ij))
                + (1-beta_i) * sum_j((1-t_ij) * ln(1-p_ij)) ]
beta_i = 1 - mean_j(t_ij)

Per-core row statistics (8 batch rows per core):
  S=sum(t)  A=sum(t*lnp)  C=sum(t*ln1mp)  B=sum(ln1mp)
host combines: loss = -sum_rows[ beta*A + (1-beta)*(B-C) ], beta = 1-S/N

Engine assignment per row tile [128, 2048]:
  - ACT: lnp = Ln(p) bf16; ln1mp = Ln(1-p) bf16 with accum_out -> B per partition
  - DVE: cast t->bf16 (2x mode); m1 = t*lnp, m2 = t*ln1mp (bf16 TT, 2x mode)
  - PE: ones-matmul chunk reductions of m1/m2 and tb (bf16), plus one
        tiny matmul for the cross-partition reduce of B
"""

from contextlib import ExitStack

import numpy as np

import concourse.bass as bass
import concourse.mybir as mybir
import concourse.tile as tile
from concourse import bacc
from concourse.bass_utils import run_bass_kernel_spmd

B, N = 64, 262144
NCORES = 8
ROWS = B // NCORES  # rows per core
P = 128  # SBUF partitions

AF = mybir.ActivationFunctionType
ALU = mybir.AluOpType
f32 = mybir.dt.float32
bf16 = mybir.dt.bfloat16

# test.py can flip this to capture an NTFF profile of the run
TRACE = False
LAST = None  # BassKernelResults of the most recent kernel() call


def _emit(tc, out_ap, inp_ap, tgt_ap, rows, n):
    """Emit the per-core program. out_ap: [1, 4*rows] f32 = [S.., B.., A.., C..]."""
    nc = tc.nc
    F = n // P
    CH = 128  # matmul moving-dim chunk; per-row PSUM stripe is CH wide
    nch = F // CH
    assert nch * CH == F

    with ExitStack() as ctx:
        io_pool = ctx.enter_context(tc.tile_pool(name="io", bufs=6))
        bf_pool = ctx.enter_context(tc.tile_pool(name="bf", bufs=3))
        psum_pool = ctx.enter_context(tc.tile_pool(name="ps", bufs=1, space="PSUM"))
        singles = ctx.enter_context(tc.tile_pool(name="const", bufs=1))

        ones_bf = singles.tile([P, 1], bf16, tag="ones_bf")
        nc.vector.memset(ones_bf[:], 1.0)
        ones_f = singles.tile([P, 1], f32, tag="ones_f")
        nc.vector.memset(ones_f[:], 1.0)
        accB = singles.tile([P, rows], f32, tag="accB")
        stats = singles.tile([1, 4 * rows], f32, tag="stats")

        inp3 = inp_ap.rearrange("r (p f) -> p r f", p=P)
        tgt3 = tgt_ap.rearrange("r (p f) -> p r f", p=P)

        # psA/psC/psS: 2 PSUM banks each; psB gets its own bank
        psA = psum_pool.tile([1, rows * CH], f32, tag="psA", name="psA")
        psC = psum_pool.tile([1, rows * CH], f32, tag="psC", name="psC")
        psS = psum_pool.tile([1, rows * CH], f32, tag="psS", name="psS")
        psB = psum_pool.tile([1, rows], f32, tag="psB", name="psB")

        # per-row 1MB loads on the otherwise-idle SP engine, all triggers
        # emitted upfront (first io_bufs rows stream immediately; later
        # triggers wait inline on slot recycling, which only stalls SP).
        # The last t row is split in half so its consumer chain starts
        # ~1us earlier.
        ptiles, ttiles = [], []
        for r in range(rows):
            pp = io_pool.tile([P, F], f32, tag="p", name=f"pp_{r}")
            nc.sync.dma_start(pp[:], inp3[:, r, :])
            ptiles.append(pp)
            tt = io_pool.tile([P, F], f32, tag="t", name=f"tt_{r}")
            nc.sync.dma_start(tt[:], tgt3[:, r, :])
            ttiles.append(tt)

        for r in range(rows):
            p_t = ptiles[r][:]
            t_t = ttiles[r][:]

            logp = bf_pool.tile([P, F], bf16, tag="logp")
            nc.scalar.activation(logp[:], p_t, AF.Ln)
            l1mp = bf_pool.tile([P, F], bf16, tag="l1mp")
            nc.scalar.activation(
                l1mp[:], p_t, AF.Ln, scale=-1.0, bias=1.0,
                accum_out=accB[:, r : r + 1],
            )

            tb = bf_pool.tile([P, F], bf16, tag="tb")
            nc.vector.tensor_copy(tb[:], t_t)
            m1 = bf_pool.tile([P, F], bf16, tag="m1")
            nc.vector.tensor_mul(m1[:], tb[:], logp[:])
            m2 = bf_pool.tile([P, F], bf16, tag="m2")
            nc.vector.tensor_mul(m2[:], tb[:], l1mp[:])

            for ps, src in ((psS, tb), (psA, m1), (psC, m2)):
                for c in range(nch):
                    nc.tensor.matmul(
                        ps[0:1, r * CH : (r + 1) * CH],
                        ones_bf[:],
                        src[:, c * CH : (c + 1) * CH],
                        start=(c == 0),
                        stop=(c == nch - 1),
                    )

            # per-row second-level reduce overlaps with later rows' stream
            for ps, col in ((psS, r), (psA, 2 * rows + r), (psC, 3 * rows + r)):
                nc.vector.tensor_reduce(
                    stats[0:1, col : col + 1],
                    ps[0:1, r * CH : (r + 1) * CH],
                    axis=mybir.AxisListType.X,
                    op=ALU.add,
                )

        # cross-partition reduce of B accumulators on PE
        nc.tensor.matmul(psB[0:1, :], ones_f[:], accB[:, :])
        nc.vector.tensor_copy(stats[0:1, rows : 2 * rows], psB[0:1, :])
        nc.sync.dma_start(out_ap, stats[:])


_PROG_CACHE = {}


def _build_program(rows=ROWS, n=N):
    key = (rows, n)
    if key not in _PROG_CACHE:
        nc = bacc.Bacc("TRN2", target_bir_lowering=False, debug=False)
        inp = nc.dram_tensor("input", [rows, n], f32, kind="ExternalInput").ap()
        tgt = nc.dram_tensor("target", [rows, n], f32, kind="ExternalInput").ap()
        out = nc.dram_tensor("partials", [1, 4 * rows], f32, kind="ExternalOutput").ap()
        with tile.TileContext(nc) as tc:
            _emit(tc, out, inp, tgt, rows, n)
        nc.finalize()
        _PROG_CACHE[key] = nc
    return _PROG_CACHE[key]


def kernel(input, target):
    global LAST
    input = np.ascontiguousarray(np.asarray(input))
    target = np.ascontiguousarray(np.asarray(target))
    assert input.shape == (B, N) and target.shape == (B, N)

    nc = _build_program()
    in_maps = [
        {
            "input": input[c * ROWS : (c + 1) * ROWS],
            "target": target[c * ROWS : (c + 1) * ROWS],
        }
        for c in range(NCORES)
    ]
    res = run_bass_kernel_spmd(nc, in_maps, core_ids=list(range(NCORES)), trace=TRACE)
    LAST = res

    total = np.float64(0.0)
    for c in range(NCORES):
        part = res.results[c]["partials"].astype(np.float64).reshape(4, ROWS)
        S, Bv, A, C = part[0], part[1], part[2], part[3]
        beta = 1.0 - S / N
        total += np.sum(beta * A + (1.0 - beta) * (Bv - C))
    return np.float32(-total)
